# revision 17
# baseline (speedup 1.0000x reference)
"""Trainium2 Bass kernel for nn_DIFF_GraphAttention (gnn_message_passing).

Math: x = tanh(features); score_e = x[col_e] @ w  (w = high - ALPHA*diff);
per-destination-row softmax over scores; out = tanh(sum_e att_e * x[col_e]).

Key identity: the segment-softmax max subtraction cancels exactly:
  att_e = exp(y[col_e]) / sum_{e' in row} exp(y[col_e'])   (y = x @ w)
so with g = exp(y) the whole computation collapses to two segment sums:
  out[r] = tanh( (sum_{e in r} g[col]*x[col]) / (sum_{e in r} g[col]) )

Structured fast path (auto-detected, else generic gather fallback):
The reference's edge list is cols[n, k] = (13 n + off_k) mod N with a fixed
offset set {off_k}.  For a 128-node destination tile and fixed k the needed
source rows are j0 + 13 i (i = 0..127) -- a stride-13 window.  Each core gets
features pre-rotated by 13*n0_core (host roll), making window coordinates
j = (1664 t + off_k) mod N identical across cores (SPMD-clean).  Phase 1
builds an SBUF-resident node table in "q-major" layout (partition = (j//13)
mod 128, per-partition slab = [qblk][j%13][x*g (128 fp16), g]), computed
directly from a 13-deinterleaved feature read, plus a wrap margin so no
window ever crosses N.  Phase 2 needs no gather and no per-block masks: for
each (tile-triple, k) a window is two PE matmuls against the table slab with
host-precomputed sliced-rotation 0/1 masks (hi: partitions >= delta shift
-delta into psum rows; lo: partitions < delta from the next q-block).  PSUM
[128, 3, 129] accumulates num|den over all 32 offsets; epilogue divides,
tanh, DMA out.  No gpsimd descriptor generation, no DVE mask builds.
"""

import os

import numpy as np

import concourse.bass as bass
import concourse.bacc as bacc
import concourse.tile as tile
from concourse import mybir
from concourse.bass_utils import run_bass_kernel_spmd
from concourse.library_config import mlp

LAST = {}  # debug: last BassKernelResults (exec_time_ns etc.)

N = 50000
D = 128
ALPHA = 0.5
NCORES = 8
NPC = N // NCORES          # nodes per core = 6250
TN = 128                   # nodes per tile
NT = (NPC + TN - 1) // TN  # tiles per core = 49
P = 128
DREAD = D + 1              # 129 floats used per table row

# ---------------- structured path constants ----------------
A_STRIDE = 13              # col stride of the structured edge pattern
QBLK = 32                  # q-blocks in the SBUF table (incl. wrap margin)
ROWLEN = 130               # fp16 slots per table row (128 x*g, 1 g, 1 pad)
CHUNK = A_STRIDE * P       # 1664 feature rows per phase-1 chunk
NQ = (N - 1) // A_STRIDE   # 3846 = max valid q
TG = 12                    # dst tiles per psum group (4 triples)

HI_BASE = 17233            # generic path: hi-window table base row
LO_MAX = 32767

TBL_KIND = os.environ.get("GNN_TBL", "fp16")
if TBL_KIND == "fp16":
    TBL_DT, TBL_NP, TBL_STRIDE = mybir.dt.float16, np.float16, 256
else:
    TBL_DT, TBL_NP, TBL_STRIDE = mybir.dt.float32, np.float32, 192
MERGE = int(os.environ.get("GNN_MERGE", "2"))  # generic: tiles per gather


# ===================== structured path =====================

def _detect_structure(adj_nei):
    """Return sorted offset list if cols[n,:] == {(13n + off) % N} else None."""
    rows = np.asarray(adj_nei[0], dtype=np.int64)
    cols = np.asarray(adj_nei[1], dtype=np.int64)
    deg, rem = divmod(len(rows), N)
    if rem != 0 or deg == 0:
        return None
    if not np.array_equal(rows, np.repeat(np.arange(N, dtype=np.int64), deg)):
        return None
    resid = (cols.reshape(N, deg)
             - A_STRIDE * np.arange(N, dtype=np.int64)[:, None]) % N
    resid.sort(axis=1)
    offs = resid[0]
    if len(np.unique(offs)) != deg:
        return None
    if not np.all(resid == offs[None, :]):
        return None
    return offs.tolist()


def _struct_schedule(offsets):
    """Per-k window runs + mask-bank contents + matmul schedule.

    Core-invariant: windows use j = (CHUNK*t + off_k) % N.
    Returns (masks, sched) where masks is [P, NM, P] fp16 and sched is a list
    of groups; each group is (tiles0, ntile, mm_list) with mm_list entries
    (triple_idx, m0, nt, mask_idx, A0, r, start, stop).
    """
    nk = len(offsets)
    # per k: tile -> (q0, r); runs of consecutive tiles with q0 step 128
    per_k = []
    for off in offsets:
        tl = []
        for t in range(NT):
            j = (CHUNK * t + off) % N
            tl.append(divmod(j, A_STRIDE))
        runs = []
        ta = 0
        for t in range(1, NT + 1):
            if (t == NT or tl[t][0] != tl[t - 1][0] + P
                    or tl[t][1] != tl[t - 1][1]):
                runs.append((ta, t - 1))
                ta = t
        per_k.append((tl, runs))

    mask_ids = {}  # (delta, kind) -> idx

    def mid(delta, kind):
        key = (delta, kind)
        if key not in mask_ids:
            mask_ids[key] = len(mask_ids)
        return mask_ids[key]

    groups = []
    g0 = 0
    while g0 < NT:
        g1 = min(NT, g0 + TG)
        # starter k: its run covers this whole group in one segment, so its
        # start=True pieces cover every psum slot exactly once.  (start=True
        # zeroes the whole PSUM bank, so only ONE start piece may touch each
        # psum tile -- a second would wipe earlier slots.)
        k_star = None
        for ki in range(nk):
            tl, runs = per_k[ki]
            if any(ra <= g0 and rb >= g1 - 1 for (ra, rb) in runs):
                k_star = ki
                break
        assert k_star is not None, "no run-clean starter k for group"
        order = [k_star] + [ki for ki in range(nk) if ki != k_star]
        mm = []
        for oi, ki in enumerate(order):
            tl, runs = per_k[ki]
            last_k = oi == nk - 1
            for (ra, rb) in runs:
                s0, s1 = max(ra, g0), min(rb, g1 - 1)
                if s0 > s1:
                    continue
                q00, r = tl[s0]
                delta = q00 % P
                hi_mm, lo_mm = [], []
                t0 = s0
                while t0 <= s1:
                    tri = (t0 - g0) // 3
                    tri_end = min(g1 - 1, g0 + tri * 3 + 2)
                    t1 = min(s1, tri_end)
                    nt = t1 - t0 + 1
                    m0 = t0 - (g0 + tri * 3)
                    A0 = tl[t0][0] // P
                    st = oi == 0
                    hi_mm.append((tri, m0, nt, mid(delta, "hi"), A0, r,
                                  st, last_k and delta == 0))
                    if delta > 0:
                        lo_mm.append((tri, m0, nt, mid(delta, "lo"), A0 + 1,
                                      r, False, last_k))
                    t0 = t1 + 1
                # all hi pieces before all lo pieces: consecutive matmuls
                # share the stationary mask (and start precedes accumulate
                # per psum slot)
                mm.extend(hi_mm)
                mm.extend(lo_mm)
        groups.append((g0, g1 - g0, mm))
        g0 = g1

    nm = len(mask_ids)
    masks = np.zeros((P, nm, P), dtype=np.float16)
    pp = np.arange(P)
    for (delta, kind), idx in mask_ids.items():
        if kind == "hi":
            sel = pp >= delta
            masks[pp[sel], idx, pp[sel] - delta] = 1.0
        else:
            sel = pp < delta
            masks[pp[sel], idx, pp[sel] + P - delta] = 1.0
    return masks, groups


def _phase1_chunks():
    """(qblk, p0, p1, r0, r1, feat_row0); table slot (qblk, p, r) takes
    feat row feat_row0 + 13*(p-p0) + (r-r0)."""
    chunks = [(qb, 0, P, 0, A_STRIDE, CHUNK * qb) for qb in range(30)]
    chunks.append((30, 0, 6, 0, A_STRIDE, 49920))   # q 3840..3845
    chunks.append((30, 6, 7, 0, 2, 49998))          # q 3846, j<N (r<2)
    chunks.append((30, 6, 7, 2, A_STRIDE, 0))       # q 3846 wrap: j-N=r-2
    chunks.append((30, 7, P, 0, A_STRIDE, 11))      # margin q 3847..3967
    # margin q 3968..3975 (p < 8); p >= 8 is never masked-in but matmuls
    # stream the whole partition range, so fill all 128 partitions with
    # finite values (uninitialized SBUF can hold fp16 NaNs; NaN*0 = NaN)
    chunks.append((31, 0, P, 0, A_STRIDE, 1584))
    return chunks


def _build_program_struct(nm, groups):
    nc = bacc.Bacc("TRN2", target_bir_lowering=False, debug=False,
                   num_devices=NCORES)
    feat = nc.dram_tensor("features", [N, D], mybir.dt.float32,
                          kind="ExternalInput").ap()
    wrep = nc.dram_tensor("wrep", [P, D], mybir.dt.float32,
                          kind="ExternalInput").ap()
    bankd = nc.dram_tensor("maskbank", [P, nm * P], mybir.dt.float16,
                           kind="ExternalInput").ap()
    out = nc.dram_tensor("out", [NPC, D], mybir.dt.float32,
                         kind="ExternalOutput").ap()
    dump = bool(int(os.environ.get("GNN_DUMP", "0")))
    if dump:
        tbl_out = nc.dram_tensor("tbl", [P, QBLK * A_STRIDE * ROWLEN],
                                 mybir.dt.float16, kind="ExternalOutput").ap()

    chunks = _phase1_chunks()

    with tile.TileContext(nc) as tc:
        with (
            tc.tile_pool(name="const", bufs=1) as cpool,
            tc.tile_pool(name="p1a", bufs=3) as p1a,
            tc.tile_pool(name="p1b", bufs=3) as p1b,
            tc.tile_pool(name="p1c", bufs=2) as p1c,
            tc.tile_pool(name="ep", bufs=3) as ep,
            tc.tile_pool(name="ps", bufs=2, space="PSUM") as psp,
        ):
            nc.gpsimd.load_library(mlp)
            table = cpool.tile([P, QBLK, A_STRIDE, ROWLEN], mybir.dt.float16)
            bank = cpool.tile([P, nm, P], mybir.dt.float16)
            wr = cpool.tile([P, D], mybir.dt.float32)
            nc.sync.dma_start(wr[:], wrep[:])
            nc.sync.dma_start(bank[:], bankd.rearrange("p (m q) -> p m q", q=P))

            # -------- Phase 1: node table (SBUF, q-major, 13-deinterleave) ----
            # compute always at partition base 0 (engine alignment rules);
            # p0 > 0 slabs go through a temp tile + partition-shifting DMA
            for (qb, p0, p1, r0, r1, row0) in chunks:
                npart = p1 - p0
                nr = r1 - r0
                fsrc = feat[row0:row0 + npart * nr].rearrange(
                    "(p r) d -> p r d", r=nr)
                ft = p1a.tile([P, A_STRIDE, D], mybir.dt.float32, tag="ft")
                nc.sync.dma_start(ft[:npart, 0:nr], fsrc)
                xt = p1b.tile([P, A_STRIDE, D], mybir.dt.float32, tag="xt")
                nc.scalar.activation(xt[:npart, 0:nr], ft[:npart, 0:nr],
                                     mybir.ActivationFunctionType.Tanh)
                tmp = p1c.tile([P, A_STRIDE, D], mybir.dt.float32, tag="tmp")
                yv = p1a.tile([P, A_STRIDE], mybir.dt.float32, tag="y")
                wap = wr[:npart, :]
                wb = bass.AP(wap.tensor, wap.offset,
                             [list(wap.ap[0]), [0, nr], list(wap.ap[1])])
                nc.vector.tensor_tensor(out=tmp[:npart, 0:nr],
                                        in0=xt[:npart, 0:nr],
                                        in1=wb, op=mybir.AluOpType.mult)
                nc.vector.tensor_reduce(out=yv[:npart, 0:nr],
                                        in_=tmp[:npart, 0:nr],
                                        axis=mybir.AxisListType.X,
                                        op=mybir.AluOpType.add)
                gv = p1b.tile([P, A_STRIDE], mybir.dt.float32, tag="g")
                nc.scalar.activation(gv[:npart, 0:nr], yv[:npart, 0:nr],
                                     mybir.ActivationFunctionType.Exp)
                if p0 == 0:
                    nc.gpsimd.tensor_tensor(
                        out=table[:npart, qb, r0:r1, 0:D], in0=xt[:npart, 0:nr],
                        in1=gv[:npart, 0:nr].to_broadcast([npart, nr, D]),
                        op=mybir.AluOpType.mult)
                    nc.vector.tensor_copy(out=table[:npart, qb, r0:r1, D],
                                          in_=gv[:npart, 0:nr])
                else:
                    xp = p1c.tile([P, A_STRIDE, DREAD], mybir.dt.float16,
                                  tag="xp")
                    nc.gpsimd.tensor_tensor(
                        out=xp[:npart, 0:nr, 0:D], in0=xt[:npart, 0:nr],
                        in1=gv[:npart, 0:nr].to_broadcast([npart, nr, D]),
                        op=mybir.AluOpType.mult)
                    nc.vector.tensor_copy(out=xp[:npart, 0:nr, D],
                                          in_=gv[:npart, 0:nr])
                    nc.sync.dma_start(table[p0:p1, qb, r0:r1, 0:DREAD],
                                      xp[:npart, 0:nr, :])

            tc.strict_bb_all_engine_barrier()
            if dump:
                nc.sync.dma_start(
                    tbl_out.rearrange("p (a s r) -> p a s r", a=QBLK,
                                      s=A_STRIDE),
                    table[:, :, :, :])

            # -------- Phase 2: rotation matmuls + epilogue ----
            for (t_first, ntile, mm) in groups:
                ntri = (ntile + 2) // 3
                psums = [psp.tile([P, 3, DREAD], mybir.dt.float32,
                                  space="PSUM", name=f"acc{j}", tag=f"acc{j}")
                         for j in range(ntri)]
                for (tri, m0, nt, mi, A0, r, st, sp) in mm:
                    nc.tensor.matmul(
                        out=psums[tri][:, m0:m0 + nt, :],
                        lhsT=bank[:, mi, :],
                        rhs=table[:, A0:A0 + nt, r, 0:DREAD],
                        start=st, stop=sp, skip_group_check=True)
                for tri in range(ntri):
                    for m in range(min(3, ntile - tri * 3)):
                        t = t_first + tri * 3 + m
                        n0 = t * TN
                        vn = min(NPC, n0 + TN) - n0
                        ps = psums[tri]
                        den = ep.tile([P, 1], mybir.dt.float32, tag="den")
                        nc.vector.tensor_scalar(
                            out=den[:], in0=ps[:, m, D:D + 1],
                            scalar1=1e-30, scalar2=None,
                            op0=mybir.AluOpType.add)
                        rec = ep.tile([P, 1], mybir.dt.float32, tag="rec")
                        nc.vector.reciprocal(rec[:], den[:])
                        ot = ep.tile([P, D], mybir.dt.float32, tag="ot")
                        nc.vector.tensor_scalar(out=ot[:], in0=ps[:, m, 0:D],
                                                scalar1=rec[:, 0:1],
                                                scalar2=None,
                                                op0=mybir.AluOpType.mult)
                        oth = ep.tile([P, D], mybir.dt.float32, tag="oth")
                        nc.scalar.activation(oth[:], ot[:],
                                             mybir.ActivationFunctionType.Tanh)
                        nc.sync.dma_start(out[n0:n0 + vn, :], oth[:vn, :])
    nc.compile()
    return nc


def _kernel_struct(features, offsets, w):
    masks, groups = _struct_schedule(offsets)
    nm = masks.shape[1]
    nc = _build_program_struct(nm, groups)
    wrep = np.tile(w[None, :], (P, 1)).astype(np.float32)
    bank = np.ascontiguousarray(masks.reshape(P, nm * P))
    in_maps = []
    for c in range(NCORES):
        off = (A_STRIDE * NPC * c) % N
        featc = np.ascontiguousarray(np.roll(features, -off, axis=0))
        in_maps.append({"features": featc, "wrep": wrep, "maskbank": bank})
    trace = bool(int(os.environ.get("GNN_TRACE", "0")))
    res = run_bass_kernel_spmd(nc, in_maps, core_ids=list(range(NCORES)),
                               trace=trace)
    LAST["res"] = res
    out = np.concatenate([res.results[c]["out"] for c in range(NCORES)],
                         axis=0)
    return out.astype(np.float32)


# ===================== generic (gather) fallback =====================

def _wrap_idx(vals):
    """Values [L] (L % 128 == 0) -> wrapped [128, L/16] int16."""
    nf = len(vals) // 16
    return np.tile(np.asarray(vals, np.int16).reshape(nf, 16).T, (8, 1))


def _host_prep(adj_nei):
    """Split edges per core/tile/window; equalize sizes across cores.

    Each (tile, window) section is padded to a whole number of 128-slot
    blocks (pad index 0 = valid row, pad seg_id -1 = masked), so sections
    can be concatenated into merged gather groups. Handles general sorted
    rows (variable degree), not just fixed degree.
    """
    rows = np.asarray(adj_nei[0], dtype=np.int64)
    cols = np.asarray(adj_nei[1], dtype=np.int64)
    raw = [[None] * NT for _ in range(NCORES)]
    node_bounds = np.searchsorted(rows, np.arange(0, N + 1, 1))
    for c in range(NCORES):
        n0c = c * NPC
        for t in range(NT):
            n0 = n0c + t * TN
            n1 = min(n0c + NPC, n0 + TN)
            e0, e1 = node_bounds[n0], node_bounds[n1]
            ct = cols[e0:e1]
            seg = rows[e0:e1] - n0  # tile-local node id, nondecreasing
            lo = ct <= LO_MAX
            raw[c][t] = (
                ct[lo].astype(np.int16), seg[lo].astype(np.int16),
                (ct[~lo] - HI_BASE).astype(np.int16), seg[~lo].astype(np.int16),
            )
    # static per-(tile, window) block counts = max across cores
    sizes = []  # [(B_lo, B_hi)] per tile
    for t in range(NT):
        llo = max(len(raw[c][t][0]) for c in range(NCORES))
        lhi = max(len(raw[c][t][2]) for c in range(NCORES))
        sizes.append((-(-llo // P) if llo else 0, -(-lhi // P) if lhi else 0))
    idx_lo, idx_hi, segs = [], [], []
    for c in range(NCORES):
        ilo_parts, ihi_parts, seg_parts = [], [], []
        for t in range(NT):
            vlo, slo, vhi, shi = raw[c][t]
            blo, bhi = sizes[t]
            for vals, sv, B, ip in ((vlo, slo, blo, ilo_parts),
                                    (vhi, shi, bhi, ihi_parts)):
                if B == 0:
                    continue
                L = B * P
                v = np.zeros(L, dtype=np.int16)  # pad idx 0: valid row, masked
                v[: len(vals)] = vals
                ip.append(_wrap_idx(v))
                s = np.full(L, -1, dtype=np.float32)
                s[: len(sv)] = sv
                seg_parts.append(s.reshape(B, P).T)  # [128, B]
        idx_lo.append(np.concatenate(ilo_parts, axis=1))
        idx_hi.append(np.concatenate(ihi_parts, axis=1))
        segs.append(np.concatenate(seg_parts, axis=1))
    return sizes, np.stack(idx_lo), np.stack(idx_hi), np.stack(segs)


def _build_program(sizes, nf_lo_tot, nf_hi_tot, totb, ablate=()):
    nc = bacc.Bacc("TRN2", target_bir_lowering=False, debug=False,
                   num_devices=NCORES)
    feat = nc.dram_tensor("features", [N, D], mybir.dt.float32,
                          kind="ExternalInput").ap()
    wrep = nc.dram_tensor("wrep", [P, D], mybir.dt.float32,
                          kind="ExternalInput").ap()
    iota = nc.dram_tensor("iota", [P, P], mybir.dt.float32,
                          kind="ExternalInput").ap()
    idxlo = nc.dram_tensor("idxlo", [P, nf_lo_tot], mybir.dt.int16,
                           kind="ExternalInput").ap()
    idxhi = nc.dram_tensor("idxhi", [P, nf_hi_tot], mybir.dt.int16,
                           kind="ExternalInput").ap()
    segsd = nc.dram_tensor("segs", [P, totb], mybir.dt.float32,
                           kind="ExternalInput").ap()
    out = nc.dram_tensor("out", [NPC, D], mybir.dt.float32,
                         kind="ExternalOutput").ap()

    AR = 8                      # feature rows per partition per phase-1 chunk
    CH = P * AR                 # 1024 rows per chunk
    NCHUNK = (N + CH - 1) // CH

    # gather groups: tiles [g*MERGE, min(NT, (g+1)*MERGE))
    groups = [list(range(g * MERGE, min(NT, (g + 1) * MERGE)))
              for g in range((NT + MERGE - 1) // MERGE)]

    with tile.TileContext(nc) as tc:
        with (
            tc.tile_pool(name="dram", bufs=1, space="DRAM") as dram_pool,
            tc.tile_pool(name="const", bufs=1) as cpool,
            tc.tile_pool(name="p2", bufs=3) as p2,
            tc.tile_pool(name="pg", bufs=(2 if MERGE >= 4 else 3)) as pg,
            tc.tile_pool(name="mk", bufs=4) as mk,
            tc.tile_pool(name="ps", bufs=2, space="PSUM") as psp,
        ):
            nc.gpsimd.load_library(mlp)
            table = dram_pool.tile([N, TBL_STRIDE], TBL_DT)
            wr = cpool.tile([P, D], mybir.dt.float32)
            io = cpool.tile([P, P], mybir.dt.float32)
            sg = cpool.tile([P, totb], mybir.dt.float32)
            ilo_sb = cpool.tile([P, nf_lo_tot], mybir.dt.int16)
            ihi_sb = cpool.tile([P, nf_hi_tot], mybir.dt.int16)
            nc.sync.dma_start(wr[:], wrep[:])
            nc.sync.dma_start(io[:], iota[:])
            nc.sync.dma_start(sg[:], segsd[:])
            nc.sync.dma_start(ilo_sb[:], idxlo[:])
            nc.sync.dma_start(ihi_sb[:], idxhi[:])

            # ---------------- Phase 1: build X' table ----------------
            with tc.tile_pool(name="p1", bufs=(2 if MERGE >= 4 else 3)) as p1:
              for ci in range(NCHUNK) if "p1" not in ablate else []:
                  r0 = ci * CH
                  r1 = min(N, r0 + CH)
                  pp = (r1 - r0) // AR
                  fsrc = feat[r0:r1].rearrange("(p a) d -> p a d", a=AR)
                  ft = p1.tile([P, AR, D], mybir.dt.float32, tag="ft")
                  nc.sync.dma_start(ft[:pp], fsrc)
                  xt = p1.tile([P, AR, D], mybir.dt.float32, tag="xt")
                  nc.scalar.activation(xt[:pp], ft[:pp],
                                       mybir.ActivationFunctionType.Tanh)
                  tmp = p1.tile([P, AR, D], mybir.dt.float32, tag="tmp")
                  yv = p1.tile([P, AR], mybir.dt.float32, tag="y")
                  wap = wr[:pp, :]
                  wb = bass.AP(wap.tensor, wap.offset,
                               [list(wap.ap[0]), [0, AR], list(wap.ap[1])])
                  nc.vector.tensor_tensor(out=tmp[:pp], in0=xt[:pp], in1=wb,
                                          op=mybir.AluOpType.mult)
                  nc.vector.tensor_reduce(out=yv[:pp], in_=tmp[:pp],
                                          axis=mybir.AxisListType.X,
                                          op=mybir.AluOpType.add)
                  gv = p1.tile([P, AR], mybir.dt.float32, tag="g")
                  nc.scalar.activation(gv[:pp], yv[:pp],
                                       mybir.ActivationFunctionType.Exp)
                  xp = p1.tile([P, AR, DREAD], TBL_DT, tag="xp")
                  nc.gpsimd.tensor_tensor(
                      out=xp[:pp, :, 0:D], in0=xt[:pp],
                      in1=gv[:pp].to_broadcast([pp, AR, D]),
                      op=mybir.AluOpType.mult)
                  nc.vector.tensor_copy(out=xp[:pp, :, D], in_=gv[:pp])
                  tdst = table[r0:r1].rearrange("(p a) s -> p a s", a=AR)
                  nc.sync.dma_start(tdst[:, :, 0:DREAD], xp[:pp])

            tc.strict_bb_all_engine_barrier()

            # ---------------- Phase 2: gather + segment sum ----------------
            flo = fhi = bo = 0
            boffs = {}  # tile -> (lo block col start, hi block col start)
            for t in range(NT):
                blo, bhi = sizes[t]
                boffs[t] = bo
                bo += blo + bhi
            # lo/hi idx + gathered-block offsets per group
            for tl in groups if "p2" not in ablate else []:
                gBlo = sum(sizes[t][0] for t in tl)
                gBhi = sum(sizes[t][1] for t in tl)
                gats = {}
                for (wname, gB, src_base, i_sb, foff) in (
                    ("lo", gBlo, 0, ilo_sb, flo),
                    ("hi", gBhi, HI_BASE, ihi_sb, fhi),
                ):
                    if gB == 0:
                        gats[wname] = None
                        continue
                    L = gB * P
                    nf = L // 16
                    gt = pg.tile([P, gB, TBL_STRIDE], TBL_DT, tag="g" + wname)
                    nc.gpsimd.dma_gather(gt[:, :gB, :], table[src_base:, :],
                                         i_sb[:, foff:foff + nf], L, L,
                                         TBL_STRIDE, single_packet=False)
                    gats[wname] = gt
                flo += gBlo * P // 16
                fhi += gBhi * P // 16

                lo_off = 0
                hi_off = 0
                for t in tl:
                    blo, bhi = sizes[t]
                    n0 = t * TN
                    vn = min(NPC, n0 + TN) - n0
                    ps = psp.tile([P, DREAD], mybir.dt.float32, space="PSUM")
                    nb_tot = blo + bhi
                    bi = 0
                    bo = boffs[t]
                    for (wname, B, off) in (("lo", blo, lo_off),
                                            ("hi", bhi, hi_off)):
                        gt = gats[wname]
                        for b in range(B):
                            if "mm" in ablate:
                                bo += 1
                                bi += 1
                                continue
                            msk = mk.tile([P, P], TBL_DT, tag="msk")
                            nc.vector.tensor_scalar(
                                out=msk[:], in0=io[:],
                                scalar1=sg[:, bo:bo + 1], scalar2=None,
                                op0=mybir.AluOpType.is_equal)
                            nc.tensor.matmul(out=ps[:], lhsT=msk[:],
                                             rhs=gt[:, off + b, 0:DREAD],
                                             start=(bi == 0),
                                             stop=(bi == nb_tot - 1))
                            bo += 1
                            bi += 1
                    lo_off += blo
                    hi_off += bhi
                    if "mm" in ablate:
                        nc.vector.memset(ps[:], 1.0)

                    den = p2.tile([P, 1], mybir.dt.float32, tag="den")
                    nc.vector.tensor_scalar(out=den[:], in0=ps[:, D:D + 1],
                                            scalar1=1e-30, scalar2=None,
                                            op0=mybir.AluOpType.add)
                    rec = p2.tile([P, 1], mybir.dt.float32, tag="rec")
                    nc.vector.reciprocal(rec[:], den[:])
                    ot = p2.tile([P, D], mybir.dt.float32, tag="ot")
                    nc.vector.tensor_scalar(out=ot[:], in0=ps[:, 0:D],
                                            scalar1=rec[:, 0:1], scalar2=None,
                                            op0=mybir.AluOpType.mult)
                    oth = p2.tile([P, D], mybir.dt.float32, tag="oth")
                    nc.scalar.activation(oth[:], ot[:],
                                         mybir.ActivationFunctionType.Tanh)
                    nc.sync.dma_start(out[n0:n0 + vn, :], oth[:vn, :])
    nc.compile()
    return nc


def _kernel_generic(features, adj_nei, w):
    sizes, idx_lo, idx_hi, segs = _host_prep(adj_nei)
    nc = _build_program(sizes, idx_lo.shape[2], idx_hi.shape[2], segs.shape[2])
    wrep = np.tile(w[None, :], (P, 1)).astype(np.float32)
    iota = np.tile(np.arange(P, dtype=np.float32)[None, :], (P, 1))
    in_maps = []
    for c in range(NCORES):
        in_maps.append({
            "features": features,
            "wrep": wrep,
            "iota": iota,
            "idxlo": np.ascontiguousarray(idx_lo[c]),
            "idxhi": np.ascontiguousarray(idx_hi[c]),
            "segs": np.ascontiguousarray(segs[c]),
        })
    trace = bool(int(os.environ.get("GNN_TRACE", "0")))
    res = run_bass_kernel_spmd(nc, in_maps, core_ids=list(range(NCORES)),
                               trace=trace)
    LAST["res"] = res
    out = np.concatenate([res.results[c]["out"] for c in range(NCORES)], axis=0)
    return out.astype(np.float32)


def kernel(features, adj_nei, high_atts, diff_atts):
    features = np.ascontiguousarray(np.asarray(features, dtype=np.float32))
    adj_nei = np.asarray(adj_nei)
    w = (np.asarray(high_atts, dtype=np.float32)[0]
         - ALPHA * np.asarray(diff_atts, dtype=np.float32)[0])

    mode = os.environ.get("GNN_MODE", "auto")
    offsets = None if mode == "generic" else _detect_structure(adj_nei)
    if offsets is not None:
        return _kernel_struct(features, offsets, w)
    return _kernel_generic(features, adj_nei, w)


# revision 19
# speedup vs baseline: 1.0930x; 1.0930x over previous
"""Trainium2 Bass kernel for nn_DIFF_GraphAttention (gnn_message_passing).

Math: x = tanh(features); score_e = x[col_e] @ w  (w = high - ALPHA*diff);
per-destination-row softmax over scores; out = tanh(sum_e att_e * x[col_e]).

Key identity: the segment-softmax max subtraction cancels exactly:
  att_e = exp(y[col_e]) / sum_{e' in row} exp(y[col_e'])   (y = x @ w)
so with g = exp(y) the whole computation collapses to two segment sums:
  out[r] = tanh( (sum_{e in r} g[col]*x[col]) / (sum_{e in r} g[col]) )

Structured fast path (auto-detected, else generic gather fallback):
The reference's edge list is cols[n, k] = (13 n + off_k) mod N with a fixed
offset set {off_k}.  For a 128-node destination tile and fixed k the needed
source rows are j0 + 13 i (i = 0..127) -- a stride-13 window.  Each core gets
features pre-rotated by 13*n0_core (host roll), making window coordinates
j = (1664 t + off_k) mod N identical across cores (SPMD-clean).  Phase 1
builds an SBUF-resident node table in "q-major" layout (partition = (j//13)
mod 128, per-partition slab = [qblk][j%13][x*g (128 fp16), g]), computed
directly from a 13-deinterleaved feature read, plus a wrap margin so no
window ever crosses N.  Phase 2 needs no gather and no per-block masks: for
each (tile-triple, k) a window is two PE matmuls against the table slab with
host-precomputed sliced-rotation 0/1 masks (hi: partitions >= delta shift
-delta into psum rows; lo: partitions < delta from the next q-block).  PSUM
[128, 3, 129] accumulates num|den over all 32 offsets; epilogue divides,
tanh, DMA out.  No gpsimd descriptor generation, no DVE mask builds.
"""

import os

import numpy as np

import concourse.bass as bass
import concourse.bacc as bacc
import concourse.tile as tile
from concourse import mybir
from concourse.bass_utils import run_bass_kernel_spmd
from concourse.library_config import mlp

LAST = {}  # debug: last BassKernelResults (exec_time_ns etc.)

N = 50000
D = 128
ALPHA = 0.5
NCORES = 8
NPC = N // NCORES          # nodes per core = 6250
TN = 128                   # nodes per tile
NT = (NPC + TN - 1) // TN  # tiles per core = 49
P = 128
DREAD = D + 1              # 129 floats used per table row

# ---------------- structured path constants ----------------
A_STRIDE = 13              # col stride of the structured edge pattern
QBLK = 32                  # q-blocks in the SBUF table (incl. wrap margin)
ROWLEN = 130               # fp16 slots per table row (128 x*g, 1 g, 1 pad)
CHUNK = A_STRIDE * P       # 1664 feature rows per phase-1 chunk
NQ = (N - 1) // A_STRIDE   # 3846 = max valid q
TG = 12                    # dst tiles per psum group (4 triples)

HI_BASE = 17233            # generic path: hi-window table base row
LO_MAX = 32767

TBL_KIND = os.environ.get("GNN_TBL", "fp16")
if TBL_KIND == "fp16":
    TBL_DT, TBL_NP, TBL_STRIDE = mybir.dt.float16, np.float16, 256
else:
    TBL_DT, TBL_NP, TBL_STRIDE = mybir.dt.float32, np.float32, 192
MERGE = int(os.environ.get("GNN_MERGE", "2"))  # generic: tiles per gather


# ===================== structured path =====================

def _detect_structure(adj_nei):
    """Return sorted offset list if cols[n,:] == {(13n + off) % N} else None."""
    rows = np.asarray(adj_nei[0], dtype=np.int64)
    cols = np.asarray(adj_nei[1], dtype=np.int64)
    deg, rem = divmod(len(rows), N)
    if rem != 0 or deg == 0:
        return None
    if not np.array_equal(rows, np.repeat(np.arange(N, dtype=np.int64), deg)):
        return None
    resid = (cols.reshape(N, deg)
             - A_STRIDE * np.arange(N, dtype=np.int64)[:, None]) % N
    resid.sort(axis=1)
    offs = resid[0]
    if len(np.unique(offs)) != deg:
        return None
    if not np.all(resid == offs[None, :]):
        return None
    return offs.tolist()


def _struct_schedule(offsets):
    """Per-k window runs + mask-bank contents + matmul schedule.

    Core-invariant: windows use j = (CHUNK*t + off_k) % N.
    Returns (masks, sched) where masks is [P, NM, P] fp16 and sched is a list
    of groups; each group is (tiles0, ntile, mm_list) with mm_list entries
    (triple_idx, m0, nt, mask_idx, A0, r, start, stop).
    """
    nk = len(offsets)
    # per k: tile -> (q0, r); runs of consecutive tiles with q0 step 128
    per_k = []
    for off in offsets:
        tl = []
        for t in range(NT):
            j = (CHUNK * t + off) % N
            tl.append(divmod(j, A_STRIDE))
        runs = []
        ta = 0
        for t in range(1, NT + 1):
            if (t == NT or tl[t][0] != tl[t - 1][0] + P
                    or tl[t][1] != tl[t - 1][1]):
                runs.append((ta, t - 1))
                ta = t
        per_k.append((tl, runs))

    mask_ids = {}  # (delta, kind) -> idx

    def mid(delta, kind):
        key = (delta, kind)
        if key not in mask_ids:
            mask_ids[key] = len(mask_ids)
        return mask_ids[key]

    groups = []
    g0 = 0
    while g0 < NT:
        g1 = min(NT, g0 + TG)
        # starter k: its run covers this whole group in one segment, so its
        # start=True pieces cover every psum slot exactly once.  (start=True
        # zeroes the whole PSUM bank, so only ONE start piece may touch each
        # psum tile -- a second would wipe earlier slots.)
        k_star = None
        for ki in range(nk):
            tl, runs = per_k[ki]
            if any(ra <= g0 and rb >= g1 - 1 for (ra, rb) in runs):
                k_star = ki
                break
        assert k_star is not None, "no run-clean starter k for group"
        order = [k_star] + [ki for ki in range(nk) if ki != k_star]
        mm = []
        for oi, ki in enumerate(order):
            tl, runs = per_k[ki]
            last_k = oi == nk - 1
            for (ra, rb) in runs:
                s0, s1 = max(ra, g0), min(rb, g1 - 1)
                if s0 > s1:
                    continue
                q00, r = tl[s0]
                delta = q00 % P
                hi_mm, lo_mm = [], []
                t0 = s0
                while t0 <= s1:
                    tri = (t0 - g0) // 3
                    tri_end = min(g1 - 1, g0 + tri * 3 + 2)
                    t1 = min(s1, tri_end)
                    nt = t1 - t0 + 1
                    m0 = t0 - (g0 + tri * 3)
                    A0 = tl[t0][0] // P
                    st = oi == 0
                    hi_mm.append((tri, m0, nt, mid(delta, "hi"), A0, r,
                                  st, last_k and delta == 0))
                    if delta > 0:
                        lo_mm.append((tri, m0, nt, mid(delta, "lo"), A0 + 1,
                                      r, False, last_k))
                    t0 = t1 + 1
                # all hi pieces before all lo pieces: consecutive matmuls
                # share the stationary mask (and start precedes accumulate
                # per psum slot)
                mm.extend(hi_mm)
                mm.extend(lo_mm)
        groups.append((g0, g1 - g0, mm))
        g0 = g1

    nm = len(mask_ids)
    masks = np.zeros((P, nm, P), dtype=np.float16)
    pp = np.arange(P)
    for (delta, kind), idx in mask_ids.items():
        if kind == "hi":
            sel = pp >= delta
            masks[pp[sel], idx, pp[sel] - delta] = 1.0
        else:
            sel = pp < delta
            masks[pp[sel], idx, pp[sel] + P - delta] = 1.0
    return masks, groups


def _phase1_chunks():
    """(qblk, p0, p1, r0, r1, feat_row0); table slot (qblk, p, r) takes
    feat row feat_row0 + 13*(p-p0) + (r-r0)."""
    chunks = [(qb, 0, P, 0, A_STRIDE, CHUNK * qb) for qb in range(30)]
    chunks.append((30, 0, 6, 0, A_STRIDE, 49920))   # q 3840..3845
    chunks.append((30, 6, 7, 0, 2, 49998))          # q 3846, j<N (r<2)
    chunks.append((30, 6, 7, 2, A_STRIDE, 0))       # q 3846 wrap: j-N=r-2
    chunks.append((30, 7, P, 0, A_STRIDE, 11))      # margin q 3847..3967
    # margin q 3968..3975 (p < 8); p >= 8 is never masked-in but matmuls
    # stream the whole partition range, so fill all 128 partitions with
    # finite values (uninitialized SBUF can hold fp16 NaNs; NaN*0 = NaN)
    chunks.append((31, 0, P, 0, A_STRIDE, 1584))
    return chunks


def _build_program_struct(nm, groups):
    nc = bacc.Bacc("TRN2", target_bir_lowering=False, debug=False,
                   num_devices=NCORES)
    feat = nc.dram_tensor("features", [N, D], mybir.dt.float32,
                          kind="ExternalInput").ap()
    wrep = nc.dram_tensor("wrep", [P, D], mybir.dt.float16,
                          kind="ExternalInput").ap()
    bankd = nc.dram_tensor("maskbank", [P, nm * P], mybir.dt.float16,
                           kind="ExternalInput").ap()
    out = nc.dram_tensor("out", [NPC, D], mybir.dt.float32,
                         kind="ExternalOutput").ap()
    dump = bool(int(os.environ.get("GNN_DUMP", "0")))
    if dump:
        tbl_out = nc.dram_tensor("tbl", [P, QBLK * A_STRIDE * ROWLEN],
                                 mybir.dt.float16, kind="ExternalOutput").ap()

    chunks = _phase1_chunks()

    with tile.TileContext(nc) as tc:
        with (
            tc.tile_pool(name="const", bufs=1) as cpool,
            tc.tile_pool(name="p1a", bufs=3) as p1a,
            tc.tile_pool(name="p1b", bufs=3) as p1b,
            tc.tile_pool(name="p1c", bufs=2) as p1c,
            tc.tile_pool(name="ep", bufs=3) as ep,
            tc.tile_pool(name="ps", bufs=2, space="PSUM") as psp,
        ):
            nc.gpsimd.load_library(mlp)
            table = cpool.tile([P, QBLK, A_STRIDE, ROWLEN], mybir.dt.float16)
            bank = cpool.tile([P, nm, P], mybir.dt.float16)
            wr = cpool.tile([P, D], mybir.dt.float16)
            nc.sync.dma_start(wr[:], wrep[:])
            nc.sync.dma_start(bank[:], bankd.rearrange("p (m q) -> p m q", q=P))

            # -------- Phase 1: node table (SBUF, q-major, 13-deinterleave) ----
            # compute always at partition base 0 (engine alignment rules);
            # p0 > 0 slabs go through a temp tile + partition-shifting DMA
            for (qb, p0, p1, r0, r1, row0) in chunks:
                npart = p1 - p0
                nr = r1 - r0
                fsrc = feat[row0:row0 + npart * nr].rearrange(
                    "(p r) d -> p r d", r=nr)
                ft = p1a.tile([P, A_STRIDE, D], mybir.dt.float32, tag="ft")
                nc.sync.dma_start(ft[:npart, 0:nr], fsrc)
                # xt = [tanh(f) (128 cols, fp16), 1.0] so the single gpsimd
                # multiply by g emits the whole 129-col table row (x*g | g)
                xt = p1b.tile([P, A_STRIDE, DREAD], mybir.dt.float16,
                              tag="xt")
                nc.scalar.activation(xt[:npart, 0:nr, 0:D], ft[:npart, 0:nr],
                                     mybir.ActivationFunctionType.Tanh)
                nc.vector.memset(xt[:npart, 0:nr, D], 1.0)
                tmp = p1c.tile([P, A_STRIDE, D], mybir.dt.float16, tag="tmp")
                yv = p1a.tile([P, A_STRIDE], mybir.dt.float32, tag="y")
                wap = wr[:npart, :]
                wb = bass.AP(wap.tensor, wap.offset,
                             [list(wap.ap[0]), [0, nr], list(wap.ap[1])])
                nc.vector.tensor_tensor(out=tmp[:npart, 0:nr],
                                        in0=xt[:npart, 0:nr, 0:D],
                                        in1=wb, op=mybir.AluOpType.mult)
                nc.vector.tensor_reduce(out=yv[:npart, 0:nr],
                                        in_=tmp[:npart, 0:nr],
                                        axis=mybir.AxisListType.X,
                                        op=mybir.AluOpType.add)
                gv = p1b.tile([P, A_STRIDE], mybir.dt.float32, tag="g")
                nc.scalar.activation(gv[:npart, 0:nr], yv[:npart, 0:nr],
                                     mybir.ActivationFunctionType.Exp)
                if p0 == 0:
                    nc.gpsimd.tensor_tensor(
                        out=table[:npart, qb, r0:r1, 0:DREAD],
                        in0=xt[:npart, 0:nr, :],
                        in1=gv[:npart, 0:nr].to_broadcast([npart, nr, DREAD]),
                        op=mybir.AluOpType.mult)
                else:
                    xp = p1c.tile([P, A_STRIDE, DREAD], mybir.dt.float16,
                                  tag="xp")
                    nc.gpsimd.tensor_tensor(
                        out=xp[:npart, 0:nr, :], in0=xt[:npart, 0:nr, :],
                        in1=gv[:npart, 0:nr].to_broadcast([npart, nr, DREAD]),
                        op=mybir.AluOpType.mult)
                    nc.sync.dma_start(table[p0:p1, qb, r0:r1, 0:DREAD],
                                      xp[:npart, 0:nr, :])

            tc.strict_bb_all_engine_barrier()
            if dump:
                nc.sync.dma_start(
                    tbl_out.rearrange("p (a s r) -> p a s r", a=QBLK,
                                      s=A_STRIDE),
                    table[:, :, :, :])

            # -------- Phase 2: rotation matmuls + epilogue ----
            for (t_first, ntile, mm) in groups:
                ntri = (ntile + 2) // 3
                psums = [psp.tile([P, 3, DREAD], mybir.dt.float32,
                                  space="PSUM", name=f"acc{j}", tag=f"acc{j}")
                         for j in range(ntri)]
                for (tri, m0, nt, mi, A0, r, st, sp) in mm:
                    nc.tensor.matmul(
                        out=psums[tri][:, m0:m0 + nt, :],
                        lhsT=bank[:, mi, :],
                        rhs=table[:, A0:A0 + nt, r, 0:DREAD],
                        start=st, stop=sp, skip_group_check=True)
                for tri in range(ntri):
                    for m in range(min(3, ntile - tri * 3)):
                        t = t_first + tri * 3 + m
                        n0 = t * TN
                        vn = min(NPC, n0 + TN) - n0
                        ps = psums[tri]
                        den = ep.tile([P, 1], mybir.dt.float32, tag="den")
                        nc.vector.tensor_scalar(
                            out=den[:], in0=ps[:, m, D:D + 1],
                            scalar1=1e-30, scalar2=None,
                            op0=mybir.AluOpType.add)
                        rec = ep.tile([P, 1], mybir.dt.float32, tag="rec")
                        nc.vector.reciprocal(rec[:], den[:])
                        ot = ep.tile([P, D], mybir.dt.float32, tag="ot")
                        nc.vector.tensor_scalar(out=ot[:], in0=ps[:, m, 0:D],
                                                scalar1=rec[:, 0:1],
                                                scalar2=None,
                                                op0=mybir.AluOpType.mult)
                        oth = ep.tile([P, D], mybir.dt.float32, tag="oth")
                        nc.scalar.activation(oth[:], ot[:],
                                             mybir.ActivationFunctionType.Tanh)
                        nc.sync.dma_start(out[n0:n0 + vn, :], oth[:vn, :])
    nc.compile()
    return nc


def _kernel_struct(features, offsets, w):
    masks, groups = _struct_schedule(offsets)
    nm = masks.shape[1]
    nc = _build_program_struct(nm, groups)
    wrep = np.tile(w[None, :], (P, 1)).astype(np.float16)
    bank = np.ascontiguousarray(masks.reshape(P, nm * P))
    in_maps = []
    for c in range(NCORES):
        off = (A_STRIDE * NPC * c) % N
        featc = np.ascontiguousarray(np.roll(features, -off, axis=0))
        in_maps.append({"features": featc, "wrep": wrep, "maskbank": bank})
    trace = bool(int(os.environ.get("GNN_TRACE", "0")))
    res = run_bass_kernel_spmd(nc, in_maps, core_ids=list(range(NCORES)),
                               trace=trace)
    LAST["res"] = res
    out = np.concatenate([res.results[c]["out"] for c in range(NCORES)],
                         axis=0)
    return out.astype(np.float32)


# ===================== generic (gather) fallback =====================

def _wrap_idx(vals):
    """Values [L] (L % 128 == 0) -> wrapped [128, L/16] int16."""
    nf = len(vals) // 16
    return np.tile(np.asarray(vals, np.int16).reshape(nf, 16).T, (8, 1))


def _host_prep(adj_nei):
    """Split edges per core/tile/window; equalize sizes across cores.

    Each (tile, window) section is padded to a whole number of 128-slot
    blocks (pad index 0 = valid row, pad seg_id -1 = masked), so sections
    can be concatenated into merged gather groups. Handles general sorted
    rows (variable degree), not just fixed degree.
    """
    rows = np.asarray(adj_nei[0], dtype=np.int64)
    cols = np.asarray(adj_nei[1], dtype=np.int64)
    raw = [[None] * NT for _ in range(NCORES)]
    node_bounds = np.searchsorted(rows, np.arange(0, N + 1, 1))
    for c in range(NCORES):
        n0c = c * NPC
        for t in range(NT):
            n0 = n0c + t * TN
            n1 = min(n0c + NPC, n0 + TN)
            e0, e1 = node_bounds[n0], node_bounds[n1]
            ct = cols[e0:e1]
            seg = rows[e0:e1] - n0  # tile-local node id, nondecreasing
            lo = ct <= LO_MAX
            raw[c][t] = (
                ct[lo].astype(np.int16), seg[lo].astype(np.int16),
                (ct[~lo] - HI_BASE).astype(np.int16), seg[~lo].astype(np.int16),
            )
    # static per-(tile, window) block counts = max across cores
    sizes = []  # [(B_lo, B_hi)] per tile
    for t in range(NT):
        llo = max(len(raw[c][t][0]) for c in range(NCORES))
        lhi = max(len(raw[c][t][2]) for c in range(NCORES))
        sizes.append((-(-llo // P) if llo else 0, -(-lhi // P) if lhi else 0))
    idx_lo, idx_hi, segs = [], [], []
    for c in range(NCORES):
        ilo_parts, ihi_parts, seg_parts = [], [], []
        for t in range(NT):
            vlo, slo, vhi, shi = raw[c][t]
            blo, bhi = sizes[t]
            for vals, sv, B, ip in ((vlo, slo, blo, ilo_parts),
                                    (vhi, shi, bhi, ihi_parts)):
                if B == 0:
                    continue
                L = B * P
                v = np.zeros(L, dtype=np.int16)  # pad idx 0: valid row, masked
                v[: len(vals)] = vals
                ip.append(_wrap_idx(v))
                s = np.full(L, -1, dtype=np.float32)
                s[: len(sv)] = sv
                seg_parts.append(s.reshape(B, P).T)  # [128, B]
        idx_lo.append(np.concatenate(ilo_parts, axis=1))
        idx_hi.append(np.concatenate(ihi_parts, axis=1))
        segs.append(np.concatenate(seg_parts, axis=1))
    return sizes, np.stack(idx_lo), np.stack(idx_hi), np.stack(segs)


def _build_program(sizes, nf_lo_tot, nf_hi_tot, totb, ablate=()):
    nc = bacc.Bacc("TRN2", target_bir_lowering=False, debug=False,
                   num_devices=NCORES)
    feat = nc.dram_tensor("features", [N, D], mybir.dt.float32,
                          kind="ExternalInput").ap()
    wrep = nc.dram_tensor("wrep", [P, D], mybir.dt.float32,
                          kind="ExternalInput").ap()
    iota = nc.dram_tensor("iota", [P, P], mybir.dt.float32,
                          kind="ExternalInput").ap()
    idxlo = nc.dram_tensor("idxlo", [P, nf_lo_tot], mybir.dt.int16,
                           kind="ExternalInput").ap()
    idxhi = nc.dram_tensor("idxhi", [P, nf_hi_tot], mybir.dt.int16,
                           kind="ExternalInput").ap()
    segsd = nc.dram_tensor("segs", [P, totb], mybir.dt.float32,
                           kind="ExternalInput").ap()
    out = nc.dram_tensor("out", [NPC, D], mybir.dt.float32,
                         kind="ExternalOutput").ap()

    AR = 8                      # feature rows per partition per phase-1 chunk
    CH = P * AR                 # 1024 rows per chunk
    NCHUNK = (N + CH - 1) // CH

    # gather groups: tiles [g*MERGE, min(NT, (g+1)*MERGE))
    groups = [list(range(g * MERGE, min(NT, (g + 1) * MERGE)))
              for g in range((NT + MERGE - 1) // MERGE)]

    with tile.TileContext(nc) as tc:
        with (
            tc.tile_pool(name="dram", bufs=1, space="DRAM") as dram_pool,
            tc.tile_pool(name="const", bufs=1) as cpool,
            tc.tile_pool(name="p2", bufs=3) as p2,
            tc.tile_pool(name="pg", bufs=(2 if MERGE >= 4 else 3)) as pg,
            tc.tile_pool(name="mk", bufs=4) as mk,
            tc.tile_pool(name="ps", bufs=2, space="PSUM") as psp,
        ):
            nc.gpsimd.load_library(mlp)
            table = dram_pool.tile([N, TBL_STRIDE], TBL_DT)
            wr = cpool.tile([P, D], mybir.dt.float32)
            io = cpool.tile([P, P], mybir.dt.float32)
            sg = cpool.tile([P, totb], mybir.dt.float32)
            ilo_sb = cpool.tile([P, nf_lo_tot], mybir.dt.int16)
            ihi_sb = cpool.tile([P, nf_hi_tot], mybir.dt.int16)
            nc.sync.dma_start(wr[:], wrep[:])
            nc.sync.dma_start(io[:], iota[:])
            nc.sync.dma_start(sg[:], segsd[:])
            nc.sync.dma_start(ilo_sb[:], idxlo[:])
            nc.sync.dma_start(ihi_sb[:], idxhi[:])

            # ---------------- Phase 1: build X' table ----------------
            with tc.tile_pool(name="p1", bufs=(2 if MERGE >= 4 else 3)) as p1:
              for ci in range(NCHUNK) if "p1" not in ablate else []:
                  r0 = ci * CH
                  r1 = min(N, r0 + CH)
                  pp = (r1 - r0) // AR
                  fsrc = feat[r0:r1].rearrange("(p a) d -> p a d", a=AR)
                  ft = p1.tile([P, AR, D], mybir.dt.float32, tag="ft")
                  nc.sync.dma_start(ft[:pp], fsrc)
                  xt = p1.tile([P, AR, D], mybir.dt.float32, tag="xt")
                  nc.scalar.activation(xt[:pp], ft[:pp],
                                       mybir.ActivationFunctionType.Tanh)
                  tmp = p1.tile([P, AR, D], mybir.dt.float32, tag="tmp")
                  yv = p1.tile([P, AR], mybir.dt.float32, tag="y")
                  wap = wr[:pp, :]
                  wb = bass.AP(wap.tensor, wap.offset,
                               [list(wap.ap[0]), [0, AR], list(wap.ap[1])])
                  nc.vector.tensor_tensor(out=tmp[:pp], in0=xt[:pp], in1=wb,
                                          op=mybir.AluOpType.mult)
                  nc.vector.tensor_reduce(out=yv[:pp], in_=tmp[:pp],
                                          axis=mybir.AxisListType.X,
                                          op=mybir.AluOpType.add)
                  gv = p1.tile([P, AR], mybir.dt.float32, tag="g")
                  nc.scalar.activation(gv[:pp], yv[:pp],
                                       mybir.ActivationFunctionType.Exp)
                  xp = p1.tile([P, AR, DREAD], TBL_DT, tag="xp")
                  nc.gpsimd.tensor_tensor(
                      out=xp[:pp, :, 0:D], in0=xt[:pp],
                      in1=gv[:pp].to_broadcast([pp, AR, D]),
                      op=mybir.AluOpType.mult)
                  nc.vector.tensor_copy(out=xp[:pp, :, D], in_=gv[:pp])
                  tdst = table[r0:r1].rearrange("(p a) s -> p a s", a=AR)
                  nc.sync.dma_start(tdst[:, :, 0:DREAD], xp[:pp])

            tc.strict_bb_all_engine_barrier()

            # ---------------- Phase 2: gather + segment sum ----------------
            flo = fhi = bo = 0
            boffs = {}  # tile -> (lo block col start, hi block col start)
            for t in range(NT):
                blo, bhi = sizes[t]
                boffs[t] = bo
                bo += blo + bhi
            # lo/hi idx + gathered-block offsets per group
            for tl in groups if "p2" not in ablate else []:
                gBlo = sum(sizes[t][0] for t in tl)
                gBhi = sum(sizes[t][1] for t in tl)
                gats = {}
                for (wname, gB, src_base, i_sb, foff) in (
                    ("lo", gBlo, 0, ilo_sb, flo),
                    ("hi", gBhi, HI_BASE, ihi_sb, fhi),
                ):
                    if gB == 0:
                        gats[wname] = None
                        continue
                    L = gB * P
                    nf = L // 16
                    gt = pg.tile([P, gB, TBL_STRIDE], TBL_DT, tag="g" + wname)
                    nc.gpsimd.dma_gather(gt[:, :gB, :], table[src_base:, :],
                                         i_sb[:, foff:foff + nf], L, L,
                                         TBL_STRIDE, single_packet=False)
                    gats[wname] = gt
                flo += gBlo * P // 16
                fhi += gBhi * P // 16

                lo_off = 0
                hi_off = 0
                for t in tl:
                    blo, bhi = sizes[t]
                    n0 = t * TN
                    vn = min(NPC, n0 + TN) - n0
                    ps = psp.tile([P, DREAD], mybir.dt.float32, space="PSUM")
                    nb_tot = blo + bhi
                    bi = 0
                    bo = boffs[t]
                    for (wname, B, off) in (("lo", blo, lo_off),
                                            ("hi", bhi, hi_off)):
                        gt = gats[wname]
                        for b in range(B):
                            if "mm" in ablate:
                                bo += 1
                                bi += 1
                                continue
                            msk = mk.tile([P, P], TBL_DT, tag="msk")
                            nc.vector.tensor_scalar(
                                out=msk[:], in0=io[:],
                                scalar1=sg[:, bo:bo + 1], scalar2=None,
                                op0=mybir.AluOpType.is_equal)
                            nc.tensor.matmul(out=ps[:], lhsT=msk[:],
                                             rhs=gt[:, off + b, 0:DREAD],
                                             start=(bi == 0),
                                             stop=(bi == nb_tot - 1))
                            bo += 1
                            bi += 1
                    lo_off += blo
                    hi_off += bhi
                    if "mm" in ablate:
                        nc.vector.memset(ps[:], 1.0)

                    den = p2.tile([P, 1], mybir.dt.float32, tag="den")
                    nc.vector.tensor_scalar(out=den[:], in0=ps[:, D:D + 1],
                                            scalar1=1e-30, scalar2=None,
                                            op0=mybir.AluOpType.add)
                    rec = p2.tile([P, 1], mybir.dt.float32, tag="rec")
                    nc.vector.reciprocal(rec[:], den[:])
                    ot = p2.tile([P, D], mybir.dt.float32, tag="ot")
                    nc.vector.tensor_scalar(out=ot[:], in0=ps[:, 0:D],
                                            scalar1=rec[:, 0:1], scalar2=None,
                                            op0=mybir.AluOpType.mult)
                    oth = p2.tile([P, D], mybir.dt.float32, tag="oth")
                    nc.scalar.activation(oth[:], ot[:],
                                         mybir.ActivationFunctionType.Tanh)
                    nc.sync.dma_start(out[n0:n0 + vn, :], oth[:vn, :])
    nc.compile()
    return nc


def _kernel_generic(features, adj_nei, w):
    sizes, idx_lo, idx_hi, segs = _host_prep(adj_nei)
    nc = _build_program(sizes, idx_lo.shape[2], idx_hi.shape[2], segs.shape[2])
    wrep = np.tile(w[None, :], (P, 1)).astype(np.float32)
    iota = np.tile(np.arange(P, dtype=np.float32)[None, :], (P, 1))
    in_maps = []
    for c in range(NCORES):
        in_maps.append({
            "features": features,
            "wrep": wrep,
            "iota": iota,
            "idxlo": np.ascontiguousarray(idx_lo[c]),
            "idxhi": np.ascontiguousarray(idx_hi[c]),
            "segs": np.ascontiguousarray(segs[c]),
        })
    trace = bool(int(os.environ.get("GNN_TRACE", "0")))
    res = run_bass_kernel_spmd(nc, in_maps, core_ids=list(range(NCORES)),
                               trace=trace)
    LAST["res"] = res
    out = np.concatenate([res.results[c]["out"] for c in range(NCORES)], axis=0)
    return out.astype(np.float32)


def kernel(features, adj_nei, high_atts, diff_atts):
    features = np.ascontiguousarray(np.asarray(features, dtype=np.float32))
    adj_nei = np.asarray(adj_nei)
    w = (np.asarray(high_atts, dtype=np.float32)[0]
         - ALPHA * np.asarray(diff_atts, dtype=np.float32)[0])

    mode = os.environ.get("GNN_MODE", "auto")
    offsets = None if mode == "generic" else _detect_structure(adj_nei)
    if offsets is not None:
        return _kernel_struct(features, offsets, w)
    return _kernel_generic(features, adj_nei, w)
